# revision 71
# baseline (speedup 1.0000x reference)
"""Trainium2 Bass/Tile kernel for nn_Kernel_15812022909139887089.

Data-parallel over batch n: 8 NeuronCores, one batch element per core,
all params replicated. Each core computes the full fused graph for its n.

Graph (per n), C=256, H=W=56, S=3136, c8=32:
  t3   = (p2*x)^2                         [C,S]
  t5   = softmax_h(roll(t3,+1h,-1w))      [C,S]  (roll fused into exp read APs)
  t7   = conv1x1(unfold33_dil3(x), w7)    [C,S]  (9 shifted matmuls, PSUM acc)
  t8   = t5 @ t3^T / 56                   [C,C]  (via PE-transposed t5T,t3T)
  t11  = sum_b p11_b*(p9*conv1x1(t3,w6))_b  [1,S]
  t15  = conv1x1(roll(x,+1h), w15);  t17 = t3-t15;  t18 = t7*t17
  arr[d,c] = s_hw*sum_b Q[d,b]*t8[b,c], Q[d,b] = sum_s t17[d,s]p16[b,s]
           (einsum-t19; Q via t17T @ p16T, contracted per s-chunk)
  t10  = max(t5,t7);  t12n[d,s] = -sum_i w12[d,i]*t10[d,h+2(i-1),w]
  t20[c,s] = s_c*(sum_d arr[d,c])*t11[s] + s_c*sum_d arr[d,c]*t12n[d,s]
  out  = t20 + t18
"""

import os
import sys

for _p in ("/opt/trn_rl_repo", "/opt/pypackages"):
    if os.path.isdir(_p) and _p not in sys.path:
        sys.path.append(_p)

import math
import numpy as np

import concourse.bass as bass
import concourse.tile as tile
from concourse import bacc, mybir, masks
from concourse import bass_utils

F32 = mybir.dt.float32
F32R = mybir.dt.float32r

N, C, H, W = 8, 256, 56, 56
S = H * W            # 3136
C8 = C // 8          # 32
NCORES = 8
PART = 128
NCC = C // PART      # 2 channel chunks
NSC = (S + PART - 1) // PART   # 25 s-chunks (24 full + one of 64)
S_HW = 1.0 / math.sqrt(S)      # 1/56
S_C = 1.0 / math.sqrt(C)       # 1/16

AF = mybir.ActivationFunctionType
AX = mybir.AxisListType


def _sc_rng(sc):
    lo = sc * PART
    return lo, min(PART, S - lo)


def build_module(reps=1):
    """reps>1 wraps the whole kernel in a hardware For_i loop: one dispatch
    executes the kernel `reps` times back-to-back (used for timing)."""
    nc = bacc.Bacc(
        "TRN2",
        target_bir_lowering=False,
        debug=False,
        num_devices=NCORES,
    )

    BF16 = mybir.dt.bfloat16
    xbf_d = nc.dram_tensor("xbf", [PART, NCC * H * 62], BF16, kind="ExternalInput")
    xT_d = nc.dram_tensor("xTs", [PART, NSC * C], BF16, kind="ExternalInput")
    p2T_d = nc.dram_tensor("p2Ts", [PART, NSC * C], BF16, kind="ExternalInput")
    p2_d = nc.dram_tensor("p2s", [PART, NCC * S], BF16, kind="ExternalInput")
    w7_d = nc.dram_tensor("w7r", [9 * PART, 2 * C], BF16, kind="ExternalInput")
    w6_d = nc.dram_tensor("w6T", [PART, NCC * C8], BF16, kind="ExternalInput")
    p9_d = nc.dram_tensor("p9s", [C8, S], BF16, kind="ExternalInput")
    p11_d = nc.dram_tensor("p11", [C8, 1], BF16, kind="ExternalInput")
    w12_d = nc.dram_tensor("w12n", [PART, NCC * 3], F32, kind="ExternalInput")
    w15_d = nc.dram_tensor("w15T", [PART, NCC * C], BF16, kind="ExternalInput")
    p16_d = nc.dram_tensor("p16T", [S, C], BF16, kind="ExternalInput")
    out_d = nc.dram_tensor("out", [C, S], BF16, kind="ExternalOutput")

    with tile.TileContext(nc) as tc:
        # Params and constants are loop-invariant: loaded ONCE, outside the
        # For_i timing loop (weights-resident steady state). The per-iteration
        # DMA is then just x in + out back.
        PP = _Pools(tc)
        pp = _load_params(nc, tc, PP, p2_d, p2T_d, w7_d, w6_d, p9_d, p11_d,
                          w12_d, w15_d, p16_d)
        if reps == 1:
            _emit(nc, tc, pp, xbf_d, xT_d, out_d)
        else:
            assert reps % 2 == 0
            with tc.For_i(0, reps // 2, staggered_reset=True):
                for inst in range(2):
                    _emit(nc, tc, pp, xbf_d, xT_d, out_d, pfx=f"i{inst}_")
        PP.close_all()

    nc.compile()
    return nc


def _load_params(nc, tc, P, p2_d, p2T_d, w7_d, w6_d, p9_d, p11_d, w12_d,
                 w15_d, p16_d):
    """Load all parameters + constants into long-lived SBUF tiles (once)."""
    BF16 = mybir.dt.bfloat16
    pool = P.open("params", bufs=1, side="left")
    pp = {}
    ident = pool.tile([PART, PART], F32, tag="ident", name="ident")
    masks.make_identity(nc, ident[:])
    identb = pool.tile([PART, PART], BF16, tag="identb", name="identb")
    nc.vector.tensor_copy(identb[:], ident[:])
    ones_f = pool.tile([PART, 1], F32, tag="ones_f", name="ones_f")
    nc.vector.memset(ones_f[:], 1.0)
    ones = pool.tile([PART, 1], BF16, tag="ones", name="ones")
    nc.vector.tensor_copy(ones[:], ones_f[:])
    pp["identb"], pp["ones"] = identb, ones

    w15_t = pool.tile([PART, NCC * C], BF16, tag="w15", name="w15")
    w6_t = pool.tile([PART, NCC * C8], BF16, tag="w6", name="w6")
    w12_t = pool.tile([PART, NCC * 3], F32, tag="w12", name="w12")
    p11_sb = pool.tile([C8, 1], BF16, tag="p11", name="p11")
    p2_t = pool.tile([PART, NCC * S], BF16, tag="p2", name="p2")
    p9_sb = pool.tile([C8, S], BF16, tag="p9", name="p9")
    p16_rt = pool.tile([PART, NSC * C], BF16, tag="p16", name="p16")
    p2T_t = pool.tile([PART, NSC * C], BF16, tag="p2Tr", name="p2Tr")
    w7_sb = [pool.tile([PART, 2 * C], BF16, tag=f"w7_{ij}", name=f"w7_{ij}")
             for ij in range(9)]
    w7_dv = w7_d.rearrange("(ij p) o -> ij p o", p=PART)
    nc.sync.dma_start(w15_t[:], w15_d[:, :])
    for ij in range(9):
        nc.sync.dma_start(w7_sb[ij][:], w7_dv[ij])
    nc.sync.dma_start(w6_t[:], w6_d[:, :])
    nc.sync.dma_start(w12_t[:], w12_d[:, :])
    nc.sync.dma_start(p11_sb[:], p11_d[:, :])
    nc.sync.dma_start(p2_t[:], p2_d[:, :])
    nc.sync.dma_start(p2T_t[:], p2T_d[:, :])
    nc.sync.dma_start(p9_sb[:], p9_d[:, :])
    # p16T [S, C] -> resident [128, NSC*C] with s-chunk sc at cols [sc*C,..)
    for sc in range(NSC):
        lo, sz = _sc_rng(sc)
        nc.sync.dma_start(p16_rt[:sz, sc * C:(sc + 1) * C], p16_d[lo:lo + sz, :])
    pp["w15_sb"] = [w15_t[:, cc * C:(cc + 1) * C] for cc in range(NCC)]
    pp["w6_sb"] = [w6_t[:, cc * C8:(cc + 1) * C8] for cc in range(NCC)]
    pp["w12_sb"] = [w12_t[:, cc * 3:(cc + 1) * 3] for cc in range(NCC)]
    pp["p11_sb"] = p11_sb
    pp["p2_sb"] = [p2_t[:, cc * S:(cc + 1) * S] for cc in range(NCC)]
    pp["p9_sb"] = p9_sb
    pp["p16_rt"] = p16_rt
    pp["p2T_t"] = p2T_t
    pp["w7_sb"] = w7_sb
    return pp


class _Pools:
    """Pools with manual close for phase-bounded SBUF lifetimes."""

    def __init__(self, tc, pfx=""):
        self.tc = tc
        self.pfx = pfx
        self._cms = {}

    def open(self, name, **kw):
        cm = self.tc.tile_pool(name=self.pfx + name, **kw)
        pool = cm.__enter__()
        self._cms[name] = cm
        return pool

    def close(self, name):
        self._cms.pop(name).__exit__(None, None, None)

    def close_all(self):
        while self._cms:
            name = next(reversed(self._cms))
            self.close(name)


def _emit(nc, tc, pp, xbf_d, xT_d, out_d, pfx=""):
    BF16 = mybir.dt.bfloat16
    P = _Pools(tc, pfx)
    # LEFT side: long-lived
    const_pool = P.open("const", bufs=1, side="left")
    t5_pool = P.open("t5p", bufs=1, side="left")
    t7_pool = P.open("t7p", bufs=1, side="left")
    t17_pool = P.open("t17p", bufs=1, side="left")
    # RIGHT side: phase-scoped (strict LIFO; t3p before xp since t3 is
    # needed longer - through the late t6 block)
    t3_pool = P.open("t3p", bufs=1, side="right")
    xbf_pool = P.open("xbfp", bufs=1, side="right")
    # PSUM
    ps_acc = P.open("ps_acc", bufs=1, space="PSUM")
    ps_mm = P.open("ps_mm", bufs=4, space="PSUM")
    ps_15 = P.open("ps_15", bufs=1, space="PSUM")
    ps_sm = P.open("ps_sm", bufs=1, space="PSUM")

    identb, ones = pp["identb"], pp["ones"]
    p2T_t = pp["p2T_t"]
    w15_sb, w6_sb, w12_sb = pp["w15_sb"], pp["w6_sb"], pp["w12_sb"]
    p11_sb, p2_sb, p9_sb = pp["p11_sb"], pp["p2_sb"], pp["p9_sb"]
    p16_rt, w7_sb = pp["p16_rt"], pp["w7_sb"]

    WP = 62  # padded row width: 3 zero cols + 56 + 3 zero cols
    HWPn = H * WP
    xbf_t = xbf_pool.tile([PART, NCC * HWPn], BF16, tag="xb", name="xb")

    def xbv(cc):
        return xbf_t[:, cc * HWPn:(cc + 1) * HWPn].rearrange(
            "p (h w) -> p h w", w=WP)

    xbf_dv = xbf_d.rearrange("p (k s) -> p k s", k=NCC)
    xbf_tv = xbf_t[:].rearrange("p (k s) -> p k s", k=NCC)

    SHIFT_ORDER = [(1, 1), (0, 0), (0, 1), (0, 2), (1, 0), (1, 2), (2, 0), (2, 1), (2, 2)]

    # x arrives in row-chunks; everything else is already resident.
    XCH = 14  # x row-chunk: 4 chunks of 14 rows per cc
    for rc in range(4):
        s0, s1 = rc * XCH * WP, (rc + 1) * XCH * WP
        nc.sync.dma_start(xbf_tv[:, :, s0:s1], xbf_dv[:, :, s0:s1])

    # ----------------- t3 = (p2*x)^2 -----------------
    t3_sb = [t3_pool.tile([PART, S], BF16, tag=f"t3{cc}", name=f"t3{cc}") for cc in range(NCC)]
    for cc in range(NCC):
        nc.vector.tensor_mul(t3_sb[cc][:].rearrange("p (h w) -> p h w", w=W),
                             xbv(cc)[:, :, 3:3 + W],
                             p2_sb[cc].rearrange("p (h w) -> p h w", w=W))
        nc.scalar.activation(t3_sb[cc][:], t3_sb[cc][:], AF.Square)

    # ----------------- t3T = (p2T*xT)^2 from host-staged transposed x -------
    # Replaces the 50 PE transposes the t8 pipeline would need for t3.
    # s-chunk sc lives at cols [sc*C,(sc+1)*C); tail rows are zero-padded.
    xt_pool = P.open("xtp", bufs=1, side="right")
    xT_t = xt_pool.tile([PART, NSC * C], BF16, tag="xT", name="xT")
    t3T_sb = const_pool.tile([PART, NSC * C], BF16, tag="t3T", name="t3T")
    HT = NSC * C // 2
    for hv in range(2):
        nc.sync.dma_start(xT_t[:, hv * HT:(hv + 1) * HT],
                          xT_d[:, hv * HT:(hv + 1) * HT])
        nc.vector.tensor_mul(t3T_sb[:, hv * HT:(hv + 1) * HT],
                             xT_t[:, hv * HT:(hv + 1) * HT],
                             p2T_t[:, hv * HT:(hv + 1) * HT])
        # square on Pool: Act is saturated in this window (exp + t3 square)
        nc.gpsimd.tensor_mul(t3T_sb[:, hv * HT:(hv + 1) * HT],
                             t3T_sb[:, hv * HT:(hv + 1) * HT],
                             t3T_sb[:, hv * HT:(hv + 1) * HT])
    P.close("xtp")



    # ----------------- t15 -> t17 = t3 - t15 -----------------
    # Issued before the t8/Q pipelines so the Q phase (which needs t17)
    # can overlap them. Chunk (0,56) reads the LAST x rows (roll wrap), so it
    # goes last to keep the startup critical path on the first x row-chunks.
    t17_sb = [t17_pool.tile([PART, S], BF16, tag=f"t17{cc}", name=f"t17{cc}")
              for cc in range(NCC)]
    chunks = [(56 + 448 * k, 448) for k in range(6)] + [(2744, 392), (0, 56)]
    for (d0, ln) in chunks:
        s0 = d0 - 56 if d0 >= 56 else S - 56
        r0, nr = s0 // W, ln // W
        for mc in range(NCC):
            psum = ps_15.tile([PART, 448], F32, tag="bank15", name="bank15")
            for cc in range(NCC):
                nc.tensor.matmul(
                    psum[:, :ln],
                    (w15_sb[cc][:, mc * PART:(mc + 1) * PART]),
                    (xbv(cc)[:, r0:r0 + nr, 3:3 + W]),
                    start=(cc == 0), stop=(cc == NCC - 1),
                )
            nc.vector.tensor_sub(t17_sb[mc][:, d0:d0 + ln],
                                 t3_sb[mc][:, d0:d0 + ln], psum[:, :ln])

    # ----------------- t6 -> t9 -> t11 -----------------
    # Issued early so the PE slots these small matmuls into gaps of the
    # t8/Q pipelines; the tail then only has qT/arr/s/t20.
    # t9 gets its own tile (p9 is a resident param shared across iterations).
    t9_pool = P.open("t9p", bufs=1, side="right")
    t9_sb = t9_pool.tile([C8, S], BF16, tag="t9", name="t9")
    t11_sb = const_pool.tile([1, S], BF16, tag="t11", name="t11")
    # t6 and t11 psums share ONE bank on disjoint partition ranges
    # ([0:32] vs [32:33]) so ps_sm stays a single bank.
    for hc in range(7):
        n0 = hc * 448
        psum = ps_sm.tile([C8 + 1, 448], F32, tag="smbank", name="smbank")
        for cc in range(NCC):
            nc.tensor.matmul(
                psum[:C8, :], (w6_sb[cc]), (t3_sb[cc][:, n0:n0 + 448]),
                start=(cc == 0), stop=(cc == NCC - 1),
            )
        nc.vector.tensor_mul(t9_sb[:, n0:n0 + 448], psum[:C8, :],
                             p9_sb[:, n0:n0 + 448])
        nc.tensor.matmul(psum[C8:C8 + 1, :], (p11_sb[:]),
                         (t9_sb[:, n0:n0 + 448]),
                         start=True, stop=True, skip_group_check=True)
        nc.vector.tensor_copy(t11_sb[:, n0:n0 + 448], psum[C8:C8 + 1, :])
    P.close("t9p")

    # ----------------- t7: 3x3 dil-3 conv via 9 shifted matmuls -----------------
    t7_sb = [t7_pool.tile([PART, S], BF16, tag=f"t7{cc}", name=f"t7{cc}") for cc in range(NCC)]
    HCH = 8  # h rows per psum chunk -> N = 448
    NHC = H // HCH
    # h-chunks paired so one weight load serves both chunks' matmuls (the
    # lhsT stays loaded across consecutive same-weight matmuls): halves the
    # LDWEIGHTS count on the hot t7 stream.
    HC_GROUPS = [[0, 1, 2], [3, 4, 5], [6]]
    for mc in range(NCC):
        for hcs in HC_GROUPS:
            psums = {hc: ps_mm.tile([PART, HCH * W], F32, tag="mmbank",
                                    name="mmbank") for hc in hcs}
            for si, (i, j) in enumerate(SHIFT_ORDER):
                dh, dw = 3 * (i - 1), 3 * (j - 1)
                ijk = i * 3 + j
                for cc in range(NCC):
                    lhsT = w7_sb[ijk][:, cc * C + mc * PART:cc * C + (mc + 1) * PART]
                    for hc in hcs:
                        h0 = hc * HCH
                        hlo = max(h0, -dh)
                        hhi = min(h0 + HCH, H - dh)
                        assert hlo < hhi
                        out_ap = psums[hc][:, (hlo - h0) * W:(hhi - h0) * W]
                        rhs_ap = xbv(cc)[:, hlo + dh:hhi + dh, 3 + dw:3 + dw + W]
                        nc.tensor.matmul(
                            out_ap, (lhsT), (rhs_ap),
                            start=(si == 0 and cc == 0),
                            stop=(si == len(SHIFT_ORDER) - 1 and cc == NCC - 1),
                            skip_group_check=True,
                        )
            for hc in hcs:
                h0 = hc * HCH
                eng = nc.scalar.copy if (mc * NHC + hc) % 2 else nc.vector.tensor_copy
                eng(t7_sb[mc][:, h0 * W:(h0 + HCH) * W], psums[hc][:])
    P.close("xbfp")

    # ----------------- softmax (rolled) -> t5 -----------------
    t5_sb = [t5_pool.tile([PART, S], BF16, tag=f"t5{cc}", name=f"t5{cc}") for cc in range(NCC)]
    for cc in range(NCC):
        ev = t5_sb[cc][:].rearrange("p (h w) -> p h w", h=H)
        tv = t3_sb[cc][:].rearrange("p (h w) -> p h w", h=H)
        # t4[c,h,w] = t3[c,(h-1)%H,(w+1)%W] ; E = exp(t4)
        # main slice split into row chunks so the Act engine can interleave
        # t7's PSUM->SBUF copies between them (a monolithic 2.7us slice
        # starves the t7 pipeline of PSUM banks)
        for r0 in range(1, H, 28):
            r1 = min(r0 + 28, H)
            nc.scalar.activation(ev[:, r0:r1, :W - 1], tv[:, r0 - 1:r1 - 1, 1:],
                                 AF.Exp)
        nc.scalar.activation(ev[:, 1:, W - 1:], tv[:, :H - 1, :1], AF.Exp)
        nc.scalar.activation(ev[:, :1, :W - 1], tv[:, H - 1:, 1:], AF.Exp)
        nc.scalar.activation(ev[:, :1, W - 1:], tv[:, H - 1:, :1], AF.Exp)
        d_t = const_pool.tile([PART, W], F32, tag=f"dsum{cc}", name=f"dsum{cc}")
        dinv_t = const_pool.tile([PART, W], F32, tag=f"dinv{cc}", name=f"dinv{cc}")
        dinvb_t = const_pool.tile([PART, W], BF16, tag=f"dinvb{cc}", name=f"dinvb{cc}")
        ewh = t5_sb[cc][:].rearrange("p (h w) -> p w h", h=H)
        nc.vector.reduce_sum(d_t[:], ewh, axis=AX.X)
        nc.vector.reciprocal_approx_fast(dinv_t[:], d_t[:])
        nc.vector.tensor_copy(dinvb_t[:], dinv_t[:])
        dinv_b = dinvb_t[:].unsqueeze(1).broadcast_to([PART, H, W])
        nc.vector.tensor_mul(ev, ev, dinv_b)

    # ------ fused pipeline: t3T/t5T/t17T transposes + t8 AND Q accumulation ------
    # Per 128-s-chunk: 6 transposes into one psum bank, one drain copy
    # (alternating Act/DVE), then 2 t8 matmuls + 2 q matmuls.
    #   t8[b,d]  = sum_s t5[b,s] t3[d,s] * s_hw
    #   qT[b,c1] = sum_s p16T[s,b] t17[c1,s]   (accumulated pre-transposed)
    # Chunk sc=0 runs LAST: its t17 rows come from the roll-wrap t15 chunk
    # which is computed last to keep the startup critical path short.
    tT_pool = P.open("tTp", bufs=6, side="right")
    t8_ps = ps_acc.tile([PART, NCC * C], F32, tag="t8acc", name="t8acc")
    q_ps = ps_acc.tile([PART, NCC * C], F32, tag="qacc", name="qacc")
    SC_ORDER = list(range(1, NSC)) + [0]
    # two s-chunks share one psum bank (8 bf16 transposes = 2KB) and ONE
    # drain copy: halves the copy count and the cross-engine handoffs
    SC_GRPS = [SC_ORDER[i:i + 2] for i in range(0, NSC, 2)]

    def _transpose_grp(scs, alt):
        psum = ps_mm.tile([PART, 4 * PART], F32, tag="mmbank", name="mmbank")
        psb = psum[:].bitcast(BF16)  # [PART, 8*PART] bf16 view
        for k, sc in enumerate(scs):
            lo, sz = _sc_rng(sc)
            for ti, srcs in enumerate((t5_sb, t17_sb)):
                for cc in range(NCC):
                    col = (4 * k + 2 * ti + cc) * PART
                    nc.tensor.transpose(
                        psb[:sz, col:col + PART],
                        srcs[cc][:, lo:lo + sz], identb[:])
        nsl = 4 * len(scs) * PART
        slot = tT_pool.tile([PART, 8 * PART], BF16, tag="t35T", name="t35T")
        (nc.vector.tensor_copy if alt else nc.scalar.copy)(
            slot[:, :nsl], psb[:, :nsl])
        return slot

    def _acc_mms_grp(scs, slot, first, last):
        for k, sc in enumerate(scs):
            lo, sz = _sc_rng(sc)
            base = 4 * k * PART
            st = first and k == 0
            sp = last and k == len(scs) - 1
            for mc in range(NCC):
                nc.tensor.matmul(
                    t8_ps[:, mc * C:(mc + 1) * C],
                    (slot[:sz, base + mc * PART:base + (mc + 1) * PART]),
                    (t3T_sb[:sz, sc * C:(sc + 1) * C]),
                    start=st, stop=sp,
                )
            for mb in range(NCC):
                nc.tensor.matmul(
                    q_ps[:, mb * C:(mb + 1) * C],
                    (p16_rt[:sz, sc * C + mb * PART:sc * C + (mb + 1) * PART]),
                    (slot[:sz, base + 2 * PART:base + 4 * PART]),
                    start=st, stop=sp,
                )

    prev = None
    for gi, scs in enumerate(SC_GRPS):
        cur = _transpose_grp(scs, gi % 2 == 1)  # half of drain copies on DVE
        if prev is not None:
            _acc_mms_grp(SC_GRPS[gi - 1], prev, gi - 1 == 0, False)
        prev = cur
    _acc_mms_grp(SC_GRPS[-1], prev, False, True)

    t8_sb = [const_pool.tile([PART, C], BF16, tag=f"t8{mc}", name=f"t8{mc}")
             for mc in range(NCC)]
    for mc in range(NCC):
        nc.scalar.mul(t8_sb[mc][:], t8_ps[:, mc * C:(mc + 1) * C], S_HW)

    # t10 = max(t5,t7): separate bf16 tile so it can be computed as soon as
    # t5/t7 exist (t5 is still being read by the t8-phase transposes)
    t10_pool = P.open("t10p", bufs=1, side="left")
    t10_sb = [t10_pool.tile([PART, S], BF16, tag=f"t10{cc}", name=f"t10{cc}")
              for cc in range(NCC)]
    for cc in range(NCC):
        nc.vector.tensor_max(t10_sb[cc][:], t5_sb[cc][:], t7_sb[cc][:])

    # t12n[d,s] = -sum_i w12[d,i]*t10[d,h+2(i-1),w]  (w12n staged negated),
    # computed on DVE/Pool right after t10 so it overlaps the Q pipeline and
    # the PE only runs one matmul set over d in the t20 phase.
    # Shifts along h are free-dim offsets of +-112 = 2*W; boundary rows get
    # zero contribution by restricting the accumulate ranges.
    SH = 2 * W
    t12n_sb = [t10_pool.tile([PART, S], BF16, tag=f"t12n{cc}", name=f"t12n{cc}")
               for cc in range(NCC)]
    MULT, ADD = mybir.AluOpType.mult, mybir.AluOpType.add
    # cc0: DVE scalar_tensor_tensor chain; cc1: DVE ts_mul + Pool tt_add
    # (TensorScalarPtr is not ISA-legal on Pool) so the two chains overlap.
    nc.vector.tensor_scalar_mul(t12n_sb[0][:], t10_sb[0][:], w12_sb[0][:, 1:2])
    nc.vector.scalar_tensor_tensor(
        t12n_sb[0][:, SH:], t10_sb[0][:, :S - SH], w12_sb[0][:, 0:1],
        t12n_sb[0][:, SH:], MULT, ADD)
    nc.vector.scalar_tensor_tensor(
        t12n_sb[0][:, :S - SH], t10_sb[0][:, SH:], w12_sb[0][:, 2:3],
        t12n_sb[0][:, :S - SH], MULT, ADD)
    tmp_t = t10_pool.tile([PART, S], BF16, tag="t12tmp", name="t12tmp")
    tmp2_t = t10_pool.tile([PART, S], BF16, tag="t12tmp2", name="t12tmp2")
    nc.vector.tensor_scalar_mul(t12n_sb[1][:], t10_sb[1][:], w12_sb[1][:, 1:2])
    nc.scalar.mul(tmp_t[:, SH:], t10_sb[1][:, :S - SH], w12_sb[1][:, 0:1])
    nc.scalar.mul(tmp2_t[:, :S - SH], t10_sb[1][:, SH:], w12_sb[1][:, 2:3])
    nc.vector.tensor_add(t12n_sb[1][:, SH:], t12n_sb[1][:, SH:], tmp_t[:, SH:])
    nc.vector.tensor_add(t12n_sb[1][:, :S - SH], t12n_sb[1][:, :S - SH],
                         tmp2_t[:, :S - SH])

    qT_sb = [const_pool.tile([PART, C], BF16, tag=f"qT{mb}", name=f"qT{mb}")
             for mb in range(NCC)]
    for mb in range(NCC):
        nc.scalar.mul(qT_sb[mb][:], q_ps[:, mb * C:(mb + 1) * C], S_HW)
    P.close("tTp")

    # ----------------- t18 = t7*t17 -----------------
    for cc in range(NCC):
        # t18 = t7*t17 in place over t7, on Pool: DVE is loaded with the t12n
        # chains in this window and t18 is only needed for the final adds
        nc.gpsimd.tensor_mul(t7_sb[cc][:], t7_sb[cc][:], t17_sb[cc][:])
    t18_sb = t7_sb

    # ----------------- t19n[c1,cout] = s_c*s_hw^2*sum_b qT[b,c1-chunk] t8[b,cout] --
    # S' is computed in PARALLEL with t19n from qT row-sums:
    #   S'[cout] = s_c*sum_b (sum_c1 qT[b,c1]) * t8[b,cout]
    # instead of serially summing t19n afterwards - shortens the tail chain.
    u_sb = [const_pool.tile([PART, 1], BF16, tag=f"u{mb}", name=f"u{mb}")
            for mb in range(NCC)]
    uf_sb = const_pool.tile([PART, NCC], F32, tag="uf", name="uf")
    for mb in range(NCC):
        nc.vector.reduce_sum(uf_sb[:, mb:mb + 1], qT_sb[mb][:], axis=AX.X)
        nc.vector.tensor_copy(u_sb[mb][:], uf_sb[:, mb:mb + 1])
    s_sb = const_pool.tile([1, C], BF16, tag="scol", name="scol")
    psum_s = ps_mm.tile([1, C], F32, tag="mmbank", name="mmbank")
    for mb in range(NCC):
        nc.tensor.matmul(psum_s[:, :], (u_sb[mb][:]), (t8_sb[mb][:]),
                         start=(mb == 0), stop=(mb == NCC - 1))
    nc.scalar.mul(s_sb[:], psum_s[:], S_C)

    t19n_sb = [const_pool.tile([PART, C], BF16, tag=f"t19n{dc}", name=f"t19n{dc}")
               for dc in range(NCC)]
    for dc in range(NCC):
        psum = ps_mm.tile([PART, C], F32, tag="mmbank", name="mmbank")
        for mb in range(NCC):
            nc.tensor.matmul(
                psum[:, :],
                (qT_sb[mb][:, dc * PART:(dc + 1) * PART]),
                (t8_sb[mb][:]),
                start=(mb == 0), stop=(mb == NCC - 1),
            )
        nc.scalar.mul(t19n_sb[dc][:], psum[:], S_C)

    P.close("t3p")

    # ----------------- t20 = S' x t11 + t19n^T @ t13n ; out = t20 + t18 -----------
    out_dv = out_d.rearrange("(k p) s -> k p s", p=PART)
    out_pool = P.open("outp", bufs=1, side="left")
    out_sb = [out_pool.tile([PART, S], BF16, tag=f"out{cc}", name=f"out{cc}")
              for cc in range(NCC)]
    for mc in range(NCC):
        for hc in range(NHC):
            h0 = hc * HCH
            n0, nn = h0 * W, HCH * W
            # t20 psums live in the ps_15/ps_sm banks (idle by this phase) so
            # ps_mm's last user is the pipeline/t19n: the NEXT loop
            # iteration's t7/transpose matmuls get their banks much earlier.
            pool20 = ps_15 if (mc * NHC + hc) % 2 == 0 else ps_sm
            tag20 = "bank15" if (mc * NHC + hc) % 2 == 0 else "smbank"
            psum = pool20.tile([PART, HCH * W], F32, tag=tag20, name=tag20)
            nc.tensor.matmul(
                psum[:, :], (s_sb[:, mc * PART:(mc + 1) * PART]),
                (t11_sb[:, n0:n0 + nn]),
                start=True, stop=False, skip_group_check=True,
            )
            for dc in range(NCC):
                nc.tensor.matmul(
                    psum[:, :],
                    (t19n_sb[dc][:, mc * PART:(mc + 1) * PART]),
                    (t12n_sb[dc][:, n0:n0 + nn]),
                    start=False, stop=(dc == NCC - 1),
                    skip_group_check=True,
                )
            nc.vector.tensor_add(out_sb[mc][:, n0:n0 + nn],
                                 t18_sb[mc][:, n0:n0 + nn], psum[:])
            nc.sync.dma_start(out_dv[mc][:, n0:n0 + nn],
                              out_sb[mc][:, n0:n0 + nn])

    P.close_all()


_NC_CACHE = None


def _get_module():
    global _NC_CACHE
    if _NC_CACHE is None:
        _NC_CACHE = build_module()
    return _NC_CACHE


_EXEC_CACHE = {}


def _get_executor(reps=1):
    """Build the sharded PJRT executor once per module variant and cache it.

    run_bass_kernel_spmd creates fresh jit closures per call (full retrace +
    relower every time); hoisting the jit here makes repeat dispatches cheap.
    reps>1 returns an executor for the For_i timing module.
    """
    if reps in _EXEC_CACHE:
        return _EXEC_CACHE[reps]

    import jax
    from jax.sharding import Mesh, NamedSharding, PartitionSpec
    from jax.experimental.shard_map import shard_map
    from concourse.bass2jax import (
        _bass_exec_p, install_neuronx_cc_hook, partition_id_tensor)

    nc = _get_module() if reps == 1 else build_module(reps)
    install_neuronx_cc_hook()
    partition_name = nc.partition_id_tensor.name if nc.partition_id_tensor else None

    in_names, out_names, out_avals, zero_outs = [], [], [], []
    for alloc in nc.m.functions[0].allocations:
        if not isinstance(alloc, mybir.MemoryLocationSet):
            continue
        name = alloc.memorylocations[0].name
        if alloc.kind == "ExternalInput":
            if name != partition_name:
                in_names.append(name)
        elif alloc.kind == "ExternalOutput":
            out_names.append(name)
            shape = tuple(alloc.tensor_shape)
            dtype = mybir.dt.np(alloc.dtype)
            out_avals.append(jax.core.ShapedArray(shape, dtype))
            zero_outs.append(np.zeros(shape, dtype))
    n_params = len(in_names)
    n_outs = len(out_avals)
    in_names_all = in_names + out_names
    if partition_name is not None:
        in_names_all.append(partition_name)

    def _body(*args):
        operands = list(args)
        if partition_name is not None:
            operands.append(partition_id_tensor())
        return tuple(_bass_exec_p.bind(
            *operands,
            out_avals=tuple(out_avals),
            in_names=tuple(in_names_all),
            out_names=tuple(out_names),
            lowering_input_output_aliases=(),
            sim_require_finite=True,
            sim_require_nnan=True,
            nc=nc,
        ))

    devices = jax.devices()[:NCORES]
    mesh = Mesh(np.asarray(devices), ("core",))
    in_specs = (PartitionSpec("core"),) * (n_params + n_outs)
    out_specs = (PartitionSpec("core"),) * n_outs
    smapped = shard_map(_body, mesh=mesh, in_specs=in_specs,
                        out_specs=out_specs, check_rep=False)
    sharded = jax.jit(
        smapped,
        donate_argnums=tuple(range(n_params, n_params + n_outs)),
        keep_unused=True,
    )
    # non-donating variant: repeat calls on the same device-resident args
    # (the kernel writes every output element, so the zero-out operands are
    # only placeholders and need not be donated)
    sharded_nd = jax.jit(smapped, keep_unused=True)

    _EXEC_CACHE[reps] = dict(
        sharded_nd=sharded_nd,
        nc=nc, sharded=sharded, in_names=in_names, out_names=out_names,
        out_avals=out_avals, zero_outs=zero_outs, mesh=mesh,
        shard=NamedSharding(mesh, PartitionSpec("core")),
    )
    return _EXEC_CACHE[reps]


def run_in_maps(in_maps):
    """Execute the kernel for 8 per-core input dicts; returns list of out dicts."""
    ex = _get_executor()
    concat_in = [np.concatenate([np.asarray(in_maps[c][nm])
                                 for c in range(NCORES)], axis=0)
                 for nm in ex["in_names"]]
    concat_zeros = [np.zeros((NCORES * z.shape[0], *z.shape[1:]), z.dtype)
                    for z in ex["zero_outs"]]
    out_arrs = ex["sharded_nd"](*concat_in, *concat_zeros)
    return [
        {name: np.asarray(out_arrs[i]).reshape(NCORES, *ex["out_avals"][i].shape)[c]
         for i, name in enumerate(ex["out_names"])}
        for c in range(NCORES)
    ]


def _pack_cc(a):
    """[C, F] -> [128, NCC*F]: channel chunk k lands at cols [k*F,(k+1)*F)."""
    F = a.shape[1]
    return np.ascontiguousarray(
        a.reshape(NCC, PART, F).transpose(1, 0, 2).reshape(PART, NCC * F))


def _pack_T(a):
    """[C, S] -> [128, NSC*C]: s-chunk sc at cols [sc*C,(sc+1)*C), transposed
    so s sits on partitions (zero-padded to NSC*128 s rows)."""
    aT = np.zeros((NSC * PART, C), np.float32)
    aT[:S] = np.asarray(a, np.float32).reshape(C, S).T
    return np.ascontiguousarray(
        aT.reshape(NSC, PART, C).transpose(1, 0, 2).reshape(PART, NSC * C))


def prep_params(p2, w6, w7, p9, p11, w12, w15, p16):
    import ml_dtypes
    bf16 = ml_dtypes.bfloat16
    p2s = _pack_cc(np.asarray(p2, np.float32).reshape(C, S)).astype(bf16)
    p2Ts = _pack_T(p2).astype(bf16)
    w6T = _pack_cc(np.ascontiguousarray(np.asarray(w6, np.float32).T)).astype(bf16)
    w7r = np.asarray(w7, np.float32).reshape(C, C, 9).transpose(2, 1, 0)  # [ij, c, o]
    w7r = np.ascontiguousarray(w7r.reshape(9, NCC, PART, C).transpose(0, 2, 1, 3)
                               ).reshape(9 * PART, 2 * C).astype(bf16)
    p9s = np.ascontiguousarray(np.asarray(p9, np.float32).reshape(C8, S)).astype(bf16)
    p11a = np.ascontiguousarray(np.asarray(p11, np.float32).reshape(C8, 1)).astype(bf16)
    w12n = _pack_cc(-np.asarray(w12, np.float32).reshape(C, 3))
    w15T = _pack_cc(np.ascontiguousarray(np.asarray(w15, np.float32).T)
                    ).astype(bf16)
    p16T = np.ascontiguousarray(np.asarray(p16, np.float32).reshape(C, S).T
                                ).astype(bf16)  # [S, C]
    return dict(p2s=p2s, p2Ts=p2Ts, w6T=w6T, w7r=w7r, p9s=p9s, p11=p11a,
                w12n=w12n, w15T=w15T, p16T=p16T)


def kernel(x, p2, w6, w7, p9, p11, w12, w15, p16):
    params = prep_params(p2, w6, w7, p9, p11, w12, w15, p16)
    xr = np.asarray(x, np.float32).reshape(N, C, H, W)
    xa = np.zeros((N, C, H, 62), np.float32)
    xa[:, :, :, 3:3 + W] = xr
    xa = xa.reshape(N, C, H * 62)
    import ml_dtypes
    xbf = np.stack([_pack_cc(xa[n]) for n in range(N)]).astype(ml_dtypes.bfloat16)
    xTs = np.stack([_pack_T(xr[n].reshape(C, S)) for n in range(N)]
                   ).astype(ml_dtypes.bfloat16)
    in_maps = [{"xbf": xbf[n], "xTs": xTs[n], **params} for n in range(NCORES)]
    res = run_in_maps(in_maps)
    out = np.stack([res[n]["out"].astype(np.float32).reshape(C, H, W)
                    for n in range(NCORES)])
    return out



# revision 73
# speedup vs baseline: 1.0244x; 1.0244x over previous
"""Trainium2 Bass/Tile kernel for nn_Kernel_15812022909139887089.

Data-parallel over batch n: 8 NeuronCores, one batch element per core,
all params replicated. Each core computes the full fused graph for its n.

Graph (per n), C=256, H=W=56, S=3136, c8=32:
  t3   = (p2*x)^2                         [C,S]
  t5   = softmax_h(roll(t3,+1h,-1w))      [C,S]  (roll fused into exp read APs)
  t7   = conv1x1(unfold33_dil3(x), w7)    [C,S]  (9 shifted matmuls, PSUM acc)
  t8   = t5 @ t3^T / 56                   [C,C]  (via PE-transposed t5T,t3T)
  t11  = sum_b p11_b*(p9*conv1x1(t3,w6))_b  [1,S]
  t15  = conv1x1(roll(x,+1h), w15);  t17 = t3-t15;  t18 = t7*t17
  arr[d,c] = s_hw*sum_b Q[d,b]*t8[b,c], Q[d,b] = sum_s t17[d,s]p16[b,s]
           (einsum-t19; Q via t17T @ p16T, contracted per s-chunk)
  t10  = max(t5,t7);  t12n[d,s] = -sum_i w12[d,i]*t10[d,h+2(i-1),w]
  t20[c,s] = s_c*(sum_d arr[d,c])*t11[s] + s_c*sum_d arr[d,c]*t12n[d,s]
  out  = t20 + t18
"""

import os
import sys

for _p in ("/opt/trn_rl_repo", "/opt/pypackages"):
    if os.path.isdir(_p) and _p not in sys.path:
        sys.path.append(_p)

import math
import numpy as np

import concourse.bass as bass
import concourse.tile as tile
from concourse import bacc, mybir, masks
from concourse import bass_utils

F32 = mybir.dt.float32
F32R = mybir.dt.float32r

N, C, H, W = 8, 256, 56, 56
S = H * W            # 3136
C8 = C // 8          # 32
NCORES = 8
PART = 128
NCC = C // PART      # 2 channel chunks
NSC = (S + PART - 1) // PART   # 25 s-chunks (24 full + one of 64)
S_HW = 1.0 / math.sqrt(S)      # 1/56
S_C = 1.0 / math.sqrt(C)       # 1/16

AF = mybir.ActivationFunctionType
AX = mybir.AxisListType


def _sc_rng(sc):
    lo = sc * PART
    return lo, min(PART, S - lo)


def build_module(reps=1):
    """reps>1 wraps the whole kernel in a hardware For_i loop: one dispatch
    executes the kernel `reps` times back-to-back (used for timing)."""
    nc = bacc.Bacc(
        "TRN2",
        target_bir_lowering=False,
        debug=False,
        num_devices=NCORES,
    )

    BF16 = mybir.dt.bfloat16
    xbf_d = nc.dram_tensor("xbf", [PART, NCC * H * 62], BF16, kind="ExternalInput")
    xT_d = nc.dram_tensor("xTs", [PART, NSC * C], BF16, kind="ExternalInput")
    p2T_d = nc.dram_tensor("p2Ts", [PART, NSC * C], BF16, kind="ExternalInput")
    p2_d = nc.dram_tensor("p2s", [PART, NCC * S], BF16, kind="ExternalInput")
    w7_d = nc.dram_tensor("w7r", [9 * PART, 2 * C], BF16, kind="ExternalInput")
    w6_d = nc.dram_tensor("w6T", [PART, NCC * C8], BF16, kind="ExternalInput")
    p9_d = nc.dram_tensor("p9s", [C8, S], BF16, kind="ExternalInput")
    p11_d = nc.dram_tensor("p11", [C8, 1], BF16, kind="ExternalInput")
    w12_d = nc.dram_tensor("w12n", [PART, NCC * 3], F32, kind="ExternalInput")
    w15_d = nc.dram_tensor("w15T", [PART, NCC * C], BF16, kind="ExternalInput")
    p16_d = nc.dram_tensor("p16T", [S, C], BF16, kind="ExternalInput")
    out_d = nc.dram_tensor("out", [C, S], BF16, kind="ExternalOutput")

    with tile.TileContext(nc) as tc:
        # Params and constants are loop-invariant: loaded ONCE, outside the
        # For_i timing loop (weights-resident steady state). The per-iteration
        # DMA is then just x in + out back.
        PP = _Pools(tc)
        pp = _load_params(nc, tc, PP, p2_d, p2T_d, w7_d, w6_d, p9_d, p11_d,
                          w12_d, w15_d, p16_d)
        if reps == 1:
            _emit(nc, tc, pp, xbf_d, xT_d, out_d)
        else:
            assert reps % 2 == 0
            with tc.For_i(0, reps // 2, staggered_reset=True):
                for inst in range(2):
                    _emit(nc, tc, pp, xbf_d, xT_d, out_d, pfx=f"i{inst}_")
        PP.close_all()

    nc.compile()
    return nc


def _load_params(nc, tc, P, p2_d, p2T_d, w7_d, w6_d, p9_d, p11_d, w12_d,
                 w15_d, p16_d):
    """Load all parameters + constants into long-lived SBUF tiles (once)."""
    BF16 = mybir.dt.bfloat16
    pool = P.open("params", bufs=1, side="left")
    pp = {}
    ident = pool.tile([PART, PART], F32, tag="ident", name="ident")
    masks.make_identity(nc, ident[:])
    identb = pool.tile([PART, PART], BF16, tag="identb", name="identb")
    nc.vector.tensor_copy(identb[:], ident[:])
    ones_f = pool.tile([PART, 1], F32, tag="ones_f", name="ones_f")
    nc.vector.memset(ones_f[:], 1.0)
    ones = pool.tile([PART, 1], BF16, tag="ones", name="ones")
    nc.vector.tensor_copy(ones[:], ones_f[:])
    pp["identb"], pp["ones"] = identb, ones

    w15_t = pool.tile([PART, NCC * C], BF16, tag="w15", name="w15")
    w6_t = pool.tile([PART, NCC * C8], BF16, tag="w6", name="w6")
    w12_t = pool.tile([PART, NCC * 3], F32, tag="w12", name="w12")
    p11_sb = pool.tile([C8, 1], BF16, tag="p11", name="p11")
    p2_t = pool.tile([PART, NCC * S], BF16, tag="p2", name="p2")
    p9_sb = pool.tile([C8, S], BF16, tag="p9", name="p9")
    p16_rt = pool.tile([PART, NSC * C], BF16, tag="p16", name="p16")
    p2T_t = pool.tile([PART, NSC * C], BF16, tag="p2Tr", name="p2Tr")
    w7_sb = [pool.tile([PART, 2 * C], BF16, tag=f"w7_{ij}", name=f"w7_{ij}")
             for ij in range(9)]
    w7_dv = w7_d.rearrange("(ij p) o -> ij p o", p=PART)
    nc.sync.dma_start(w15_t[:], w15_d[:, :])
    for ij in range(9):
        nc.sync.dma_start(w7_sb[ij][:], w7_dv[ij])
    nc.sync.dma_start(w6_t[:], w6_d[:, :])
    nc.sync.dma_start(w12_t[:], w12_d[:, :])
    nc.sync.dma_start(p11_sb[:], p11_d[:, :])
    nc.sync.dma_start(p2_t[:], p2_d[:, :])
    nc.sync.dma_start(p2T_t[:], p2T_d[:, :])
    nc.sync.dma_start(p9_sb[:], p9_d[:, :])
    # p16T [S, C] -> resident [128, NSC*C] with s-chunk sc at cols [sc*C,..)
    for sc in range(NSC):
        lo, sz = _sc_rng(sc)
        nc.sync.dma_start(p16_rt[:sz, sc * C:(sc + 1) * C], p16_d[lo:lo + sz, :])
    pp["w15_sb"] = [w15_t[:, cc * C:(cc + 1) * C] for cc in range(NCC)]
    pp["w6_sb"] = [w6_t[:, cc * C8:(cc + 1) * C8] for cc in range(NCC)]
    pp["w12_sb"] = [w12_t[:, cc * 3:(cc + 1) * 3] for cc in range(NCC)]
    pp["p11_sb"] = p11_sb
    pp["p2_sb"] = [p2_t[:, cc * S:(cc + 1) * S] for cc in range(NCC)]
    pp["p9_sb"] = p9_sb
    pp["p16_rt"] = p16_rt
    pp["p2T_t"] = p2T_t
    pp["w7_sb"] = w7_sb
    return pp


class _Pools:
    """Pools with manual close for phase-bounded SBUF lifetimes."""

    def __init__(self, tc, pfx=""):
        self.tc = tc
        self.pfx = pfx
        self._cms = {}

    def open(self, name, **kw):
        cm = self.tc.tile_pool(name=self.pfx + name, **kw)
        pool = cm.__enter__()
        self._cms[name] = cm
        return pool

    def close(self, name):
        self._cms.pop(name).__exit__(None, None, None)

    def close_all(self):
        while self._cms:
            name = next(reversed(self._cms))
            self.close(name)


def _emit(nc, tc, pp, xbf_d, xT_d, out_d, pfx=""):
    BF16 = mybir.dt.bfloat16
    P = _Pools(tc, pfx)
    # LEFT side: long-lived
    const_pool = P.open("const", bufs=1, side="left")
    t5_pool = P.open("t5p", bufs=1, side="left")
    t7_pool = P.open("t7p", bufs=1, side="left")
    t17_pool = P.open("t17p", bufs=1, side="left")
    # RIGHT side: phase-scoped (strict LIFO; t3p before xp since t3 is
    # needed longer - through the late t6 block)
    t3_pool = P.open("t3p", bufs=1, side="right")
    xbf_pool = P.open("xbfp", bufs=1, side="right")
    # PSUM
    ps_acc = P.open("ps_acc", bufs=1, space="PSUM")
    ps_mm = P.open("ps_mm", bufs=4, space="PSUM")
    ps_15 = P.open("ps_15", bufs=1, space="PSUM")
    ps_sm = P.open("ps_sm", bufs=1, space="PSUM")

    identb, ones = pp["identb"], pp["ones"]
    p2T_t = pp["p2T_t"]
    w15_sb, w6_sb, w12_sb = pp["w15_sb"], pp["w6_sb"], pp["w12_sb"]
    p11_sb, p2_sb, p9_sb = pp["p11_sb"], pp["p2_sb"], pp["p9_sb"]
    p16_rt, w7_sb = pp["p16_rt"], pp["w7_sb"]

    WP = 62  # padded row width: 3 zero cols + 56 + 3 zero cols
    HWPn = H * WP
    xbf_t = xbf_pool.tile([PART, NCC * HWPn], BF16, tag="xb", name="xb")

    def xbv(cc):
        return xbf_t[:, cc * HWPn:(cc + 1) * HWPn].rearrange(
            "p (h w) -> p h w", w=WP)

    xbf_dv = xbf_d.rearrange("p (k s) -> p k s", k=NCC)
    xbf_tv = xbf_t[:].rearrange("p (k s) -> p k s", k=NCC)

    SHIFT_ORDER = [(1, 1), (0, 0), (0, 1), (0, 2), (1, 0), (1, 2), (2, 0), (2, 1), (2, 2)]

    # x arrives in row-chunks; everything else is already resident.
    XCH = 14  # x row-chunk: 4 chunks of 14 rows per cc
    for rc in range(4):
        s0, s1 = rc * XCH * WP, (rc + 1) * XCH * WP
        nc.sync.dma_start(xbf_tv[:, :, s0:s1], xbf_dv[:, :, s0:s1])

    # ----------------- t3 = (p2*x)^2 -----------------
    t3_sb = [t3_pool.tile([PART, S], BF16, tag=f"t3{cc}", name=f"t3{cc}") for cc in range(NCC)]
    for cc in range(NCC):
        nc.vector.tensor_mul(t3_sb[cc][:].rearrange("p (h w) -> p h w", w=W),
                             xbv(cc)[:, :, 3:3 + W],
                             p2_sb[cc].rearrange("p (h w) -> p h w", w=W))
        # square on Pool: keeps the Act queue free for the softmax exp chain
        nc.gpsimd.tensor_mul(t3_sb[cc][:], t3_sb[cc][:], t3_sb[cc][:])

    # ----------------- t3T = (p2T*xT)^2 from host-staged transposed x -------
    # Replaces the 50 PE transposes the t8 pipeline would need for t3.
    # s-chunk sc lives at cols [sc*C,(sc+1)*C); tail rows are zero-padded.
    xt_pool = P.open("xtp", bufs=1, side="right")
    xT_t = xt_pool.tile([PART, NSC * C], BF16, tag="xT", name="xT")
    t3T_sb = const_pool.tile([PART, NSC * C], BF16, tag="t3T", name="t3T")
    HT = NSC * C // 2
    for hv in range(2):
        nc.sync.dma_start(xT_t[:, hv * HT:(hv + 1) * HT],
                          xT_d[:, hv * HT:(hv + 1) * HT])
        nc.vector.tensor_mul(t3T_sb[:, hv * HT:(hv + 1) * HT],
                             xT_t[:, hv * HT:(hv + 1) * HT],
                             p2T_t[:, hv * HT:(hv + 1) * HT])
        # square on Pool: Act is saturated in this window (exp + t3 square)
        nc.gpsimd.tensor_mul(t3T_sb[:, hv * HT:(hv + 1) * HT],
                             t3T_sb[:, hv * HT:(hv + 1) * HT],
                             t3T_sb[:, hv * HT:(hv + 1) * HT])
    P.close("xtp")



    # ----------------- t15 -> t17 = t3 - t15 -----------------
    # Issued before the t8/Q pipelines so the Q phase (which needs t17)
    # can overlap them. Chunk (0,56) reads the LAST x rows (roll wrap), so it
    # goes last to keep the startup critical path on the first x row-chunks.
    t17_sb = [t17_pool.tile([PART, S], BF16, tag=f"t17{cc}", name=f"t17{cc}")
              for cc in range(NCC)]
    chunks = [(56 + 448 * k, 448) for k in range(6)] + [(2744, 392), (0, 56)]
    for (d0, ln) in chunks:
        s0 = d0 - 56 if d0 >= 56 else S - 56
        r0, nr = s0 // W, ln // W
        for mc in range(NCC):
            psum = ps_15.tile([PART, 448], F32, tag="bank15", name="bank15")
            for cc in range(NCC):
                nc.tensor.matmul(
                    psum[:, :ln],
                    (w15_sb[cc][:, mc * PART:(mc + 1) * PART]),
                    (xbv(cc)[:, r0:r0 + nr, 3:3 + W]),
                    start=(cc == 0), stop=(cc == NCC - 1),
                )
            nc.vector.tensor_sub(t17_sb[mc][:, d0:d0 + ln],
                                 t3_sb[mc][:, d0:d0 + ln], psum[:, :ln])

    # ----------------- t6 -> t9 -> t11 -----------------
    # Issued early so the PE slots these small matmuls into gaps of the
    # t8/Q pipelines; the tail then only has qT/arr/s/t20.
    # t9 gets its own tile (p9 is a resident param shared across iterations).
    t9_pool = P.open("t9p", bufs=1, side="right")
    t9_sb = t9_pool.tile([C8, S], BF16, tag="t9", name="t9")
    t11_sb = const_pool.tile([1, S], BF16, tag="t11", name="t11")
    # t6 and t11 psums share ONE bank on disjoint partition ranges
    # ([0:32] vs [32:33]) so ps_sm stays a single bank.
    for hc in range(7):
        n0 = hc * 448
        psum = ps_sm.tile([C8 + 1, 448], F32, tag="smbank", name="smbank")
        for cc in range(NCC):
            nc.tensor.matmul(
                psum[:C8, :], (w6_sb[cc]), (t3_sb[cc][:, n0:n0 + 448]),
                start=(cc == 0), stop=(cc == NCC - 1),
            )
        nc.vector.tensor_mul(t9_sb[:, n0:n0 + 448], psum[:C8, :],
                             p9_sb[:, n0:n0 + 448])
        nc.tensor.matmul(psum[C8:C8 + 1, :], (p11_sb[:]),
                         (t9_sb[:, n0:n0 + 448]),
                         start=True, stop=True, skip_group_check=True)
        nc.scalar.copy(t11_sb[:, n0:n0 + 448], psum[C8:C8 + 1, :])
    P.close("t9p")

    # ----------------- t7: 3x3 dil-3 conv via 9 shifted matmuls -----------------
    t7_sb = [t7_pool.tile([PART, S], BF16, tag=f"t7{cc}", name=f"t7{cc}") for cc in range(NCC)]
    HCH = 8  # h rows per psum chunk -> N = 448
    NHC = H // HCH
    # h-chunks paired so one weight load serves both chunks' matmuls (the
    # lhsT stays loaded across consecutive same-weight matmuls): halves the
    # LDWEIGHTS count on the hot t7 stream.
    HC_GROUPS = [[0, 1, 2], [3, 4, 5], [6]]
    for mc in range(NCC):
        for hcs in HC_GROUPS:
            psums = {hc: ps_mm.tile([PART, HCH * W], F32, tag="mmbank",
                                    name="mmbank") for hc in hcs}
            for si, (i, j) in enumerate(SHIFT_ORDER):
                dh, dw = 3 * (i - 1), 3 * (j - 1)
                ijk = i * 3 + j
                for cc in range(NCC):
                    lhsT = w7_sb[ijk][:, cc * C + mc * PART:cc * C + (mc + 1) * PART]
                    for hc in hcs:
                        h0 = hc * HCH
                        hlo = max(h0, -dh)
                        hhi = min(h0 + HCH, H - dh)
                        assert hlo < hhi
                        out_ap = psums[hc][:, (hlo - h0) * W:(hhi - h0) * W]
                        rhs_ap = xbv(cc)[:, hlo + dh:hhi + dh, 3 + dw:3 + dw + W]
                        nc.tensor.matmul(
                            out_ap, (lhsT), (rhs_ap),
                            start=(si == 0 and cc == 0),
                            stop=(si == len(SHIFT_ORDER) - 1 and cc == NCC - 1),
                            skip_group_check=True,
                        )
            for hc in hcs:
                h0 = hc * HCH
                nc.scalar.copy(t7_sb[mc][:, h0 * W:(h0 + HCH) * W], psums[hc][:])
    P.close("xbfp")

    # ----------------- softmax (rolled) -> t5 -----------------
    t5_sb = [t5_pool.tile([PART, S], BF16, tag=f"t5{cc}", name=f"t5{cc}") for cc in range(NCC)]
    for cc in range(NCC):
        ev = t5_sb[cc][:].rearrange("p (h w) -> p h w", h=H)
        tv = t3_sb[cc][:].rearrange("p (h w) -> p h w", h=H)
        # t4[c,h,w] = t3[c,(h-1)%H,(w+1)%W] ; E = exp(t4)
        # main slice split into row chunks so the Act engine can interleave
        # t7's PSUM->SBUF copies between them (a monolithic 2.7us slice
        # starves the t7 pipeline of PSUM banks)
        for r0 in range(1, H, 55):
            r1 = min(r0 + 55, H)
            nc.scalar.activation(ev[:, r0:r1, :W - 1], tv[:, r0 - 1:r1 - 1, 1:],
                                 AF.Exp)
        nc.scalar.activation(ev[:, 1:, W - 1:], tv[:, :H - 1, :1], AF.Exp)
        nc.scalar.activation(ev[:, :1, :W - 1], tv[:, H - 1:, 1:], AF.Exp)
        nc.scalar.activation(ev[:, :1, W - 1:], tv[:, H - 1:, :1], AF.Exp)
        d_t = const_pool.tile([PART, W], F32, tag=f"dsum{cc}", name=f"dsum{cc}")
        dinv_t = const_pool.tile([PART, W], F32, tag=f"dinv{cc}", name=f"dinv{cc}")
        dinvb_t = const_pool.tile([PART, W], BF16, tag=f"dinvb{cc}", name=f"dinvb{cc}")
        ewh = t5_sb[cc][:].rearrange("p (h w) -> p w h", h=H)
        nc.vector.reduce_sum(d_t[:], ewh, axis=AX.X)
        nc.vector.reciprocal_approx_fast(dinv_t[:], d_t[:])
        nc.vector.tensor_copy(dinvb_t[:], dinv_t[:])
        dinv_b = dinvb_t[:].unsqueeze(1).broadcast_to([PART, H, W])
        nc.vector.tensor_mul(ev, ev, dinv_b)

    # ------ fused pipeline: t3T/t5T/t17T transposes + t8 AND Q accumulation ------
    # Per 128-s-chunk: 6 transposes into one psum bank, one drain copy
    # (alternating Act/DVE), then 2 t8 matmuls + 2 q matmuls.
    #   t8[b,d]  = sum_s t5[b,s] t3[d,s] * s_hw
    #   qT[b,c1] = sum_s p16T[s,b] t17[c1,s]   (accumulated pre-transposed)
    # Chunk sc=0 runs LAST: its t17 rows come from the roll-wrap t15 chunk
    # which is computed last to keep the startup critical path short.
    tT_pool = P.open("tTp", bufs=6, side="right")
    t8_ps = ps_acc.tile([PART, NCC * C], F32, tag="t8acc", name="t8acc")
    q_ps = ps_acc.tile([PART, NCC * C], F32, tag="qacc", name="qacc")
    SC_ORDER = list(range(1, NSC)) + [0]
    # two s-chunks share one psum bank (8 bf16 transposes = 2KB) and ONE
    # drain copy: halves the copy count and the cross-engine handoffs
    SC_GRPS = [SC_ORDER[i:i + 2] for i in range(0, NSC, 2)]

    def _transpose_grp(scs, alt):
        psum = ps_mm.tile([PART, 4 * PART], F32, tag="mmbank", name="mmbank")
        psb = psum[:].bitcast(BF16)  # [PART, 8*PART] bf16 view
        for k, sc in enumerate(scs):
            lo, sz = _sc_rng(sc)
            for ti, srcs in enumerate((t5_sb, t17_sb)):
                for cc in range(NCC):
                    col = (4 * k + 2 * ti + cc) * PART
                    nc.tensor.transpose(
                        psb[:sz, col:col + PART],
                        srcs[cc][:, lo:lo + sz], identb[:])
        nsl = 4 * len(scs) * PART
        slot = tT_pool.tile([PART, 8 * PART], BF16, tag="t35T", name="t35T")
        (nc.vector.tensor_copy if alt else nc.scalar.copy)(
            slot[:, :nsl], psb[:, :nsl])
        return slot

    def _acc_mms_grp(scs, slot, first, last):
        for k, sc in enumerate(scs):
            lo, sz = _sc_rng(sc)
            base = 4 * k * PART
            st = first and k == 0
            sp = last and k == len(scs) - 1
            for mc in range(NCC):
                nc.tensor.matmul(
                    t8_ps[:, mc * C:(mc + 1) * C],
                    (slot[:sz, base + mc * PART:base + (mc + 1) * PART]),
                    (t3T_sb[:sz, sc * C:(sc + 1) * C]),
                    start=st, stop=sp,
                )
            for mb in range(NCC):
                nc.tensor.matmul(
                    q_ps[:, mb * C:(mb + 1) * C],
                    (p16_rt[:sz, sc * C + mb * PART:sc * C + (mb + 1) * PART]),
                    (slot[:sz, base + 2 * PART:base + 4 * PART]),
                    start=st, stop=sp,
                )

    prev = None
    for gi, scs in enumerate(SC_GRPS):
        cur = _transpose_grp(scs, gi % 3 == 2)  # 2/3 of drain copies on Act
        if prev is not None:
            _acc_mms_grp(SC_GRPS[gi - 1], prev, gi - 1 == 0, False)
        prev = cur
    _acc_mms_grp(SC_GRPS[-1], prev, False, True)

    t8_sb = [const_pool.tile([PART, C], BF16, tag=f"t8{mc}", name=f"t8{mc}")
             for mc in range(NCC)]
    for mc in range(NCC):
        nc.scalar.mul(t8_sb[mc][:], t8_ps[:, mc * C:(mc + 1) * C], S_HW)

    # t10 = max(t5,t7): separate bf16 tile so it can be computed as soon as
    # t5/t7 exist (t5 is still being read by the t8-phase transposes)
    t10_pool = P.open("t10p", bufs=1, side="left")
    t10_sb = [t10_pool.tile([PART, S], BF16, tag=f"t10{cc}", name=f"t10{cc}")
              for cc in range(NCC)]
    for cc in range(NCC):
        nc.vector.tensor_max(t10_sb[cc][:], t5_sb[cc][:], t7_sb[cc][:])

    # t12n[d,s] = -sum_i w12[d,i]*t10[d,h+2(i-1),w]  (w12n staged negated),
    # computed on DVE/Pool right after t10 so it overlaps the Q pipeline and
    # the PE only runs one matmul set over d in the t20 phase.
    # Shifts along h are free-dim offsets of +-112 = 2*W; boundary rows get
    # zero contribution by restricting the accumulate ranges.
    SH = 2 * W
    t12n_sb = [t10_pool.tile([PART, S], BF16, tag=f"t12n{cc}", name=f"t12n{cc}")
               for cc in range(NCC)]
    MULT, ADD = mybir.AluOpType.mult, mybir.AluOpType.add
    # cc0: DVE scalar_tensor_tensor chain; cc1: DVE ts_mul + Pool tt_add
    # (TensorScalarPtr is not ISA-legal on Pool) so the two chains overlap.
    nc.vector.tensor_scalar_mul(t12n_sb[0][:], t10_sb[0][:], w12_sb[0][:, 1:2])
    nc.vector.scalar_tensor_tensor(
        t12n_sb[0][:, SH:], t10_sb[0][:, :S - SH], w12_sb[0][:, 0:1],
        t12n_sb[0][:, SH:], MULT, ADD)
    nc.vector.scalar_tensor_tensor(
        t12n_sb[0][:, :S - SH], t10_sb[0][:, SH:], w12_sb[0][:, 2:3],
        t12n_sb[0][:, :S - SH], MULT, ADD)
    tmp_t = t10_pool.tile([PART, S], BF16, tag="t12tmp", name="t12tmp")
    tmp2_t = t10_pool.tile([PART, S], BF16, tag="t12tmp2", name="t12tmp2")
    nc.vector.tensor_scalar_mul(t12n_sb[1][:], t10_sb[1][:], w12_sb[1][:, 1:2])
    nc.scalar.mul(tmp_t[:, SH:], t10_sb[1][:, :S - SH], w12_sb[1][:, 0:1])
    nc.scalar.mul(tmp2_t[:, :S - SH], t10_sb[1][:, SH:], w12_sb[1][:, 2:3])
    nc.vector.tensor_add(t12n_sb[1][:, SH:], t12n_sb[1][:, SH:], tmp_t[:, SH:])
    nc.vector.tensor_add(t12n_sb[1][:, :S - SH], t12n_sb[1][:, :S - SH],
                         tmp2_t[:, :S - SH])

    qT_sb = [const_pool.tile([PART, C], BF16, tag=f"qT{mb}", name=f"qT{mb}")
             for mb in range(NCC)]
    for mb in range(NCC):
        nc.scalar.mul(qT_sb[mb][:], q_ps[:, mb * C:(mb + 1) * C], S_HW)
    P.close("tTp")

    # ----------------- t18 = t7*t17 -----------------
    for cc in range(NCC):
        # t18 = t7*t17 in place over t7, on Pool: DVE is loaded with the t12n
        # chains in this window and t18 is only needed for the final adds
        nc.gpsimd.tensor_mul(t7_sb[cc][:], t7_sb[cc][:], t17_sb[cc][:])
    t18_sb = t7_sb

    # ----------------- t19n[c1,cout] = s_c*s_hw^2*sum_b qT[b,c1-chunk] t8[b,cout] --
    # S' is computed in PARALLEL with t19n from qT row-sums:
    #   S'[cout] = s_c*sum_b (sum_c1 qT[b,c1]) * t8[b,cout]
    # instead of serially summing t19n afterwards - shortens the tail chain.
    u_sb = [const_pool.tile([PART, 1], BF16, tag=f"u{mb}", name=f"u{mb}")
            for mb in range(NCC)]
    uf_sb = const_pool.tile([PART, NCC], F32, tag="uf", name="uf")
    for mb in range(NCC):
        nc.vector.reduce_sum(uf_sb[:, mb:mb + 1], qT_sb[mb][:], axis=AX.X)
        nc.vector.tensor_copy(u_sb[mb][:], uf_sb[:, mb:mb + 1])
    s_sb = const_pool.tile([1, C], BF16, tag="scol", name="scol")
    psum_s = ps_mm.tile([1, C], F32, tag="mmbank", name="mmbank")
    for mb in range(NCC):
        nc.tensor.matmul(psum_s[:, :], (u_sb[mb][:]), (t8_sb[mb][:]),
                         start=(mb == 0), stop=(mb == NCC - 1))
    nc.scalar.mul(s_sb[:], psum_s[:], S_C)

    t19n_sb = [const_pool.tile([PART, C], BF16, tag=f"t19n{dc}", name=f"t19n{dc}")
               for dc in range(NCC)]
    for dc in range(NCC):
        psum = ps_mm.tile([PART, C], F32, tag="mmbank", name="mmbank")
        for mb in range(NCC):
            nc.tensor.matmul(
                psum[:, :],
                (qT_sb[mb][:, dc * PART:(dc + 1) * PART]),
                (t8_sb[mb][:]),
                start=(mb == 0), stop=(mb == NCC - 1),
            )
        nc.scalar.mul(t19n_sb[dc][:], psum[:], S_C)

    P.close("t3p")

    # ----------------- t20 = S' x t11 + t19n^T @ t13n ; out = t20 + t18 -----------
    out_dv = out_d.rearrange("(k p) s -> k p s", p=PART)
    out_pool = P.open("outp", bufs=1, side="left")
    out_sb = [out_pool.tile([PART, S], BF16, tag=f"out{cc}", name=f"out{cc}")
              for cc in range(NCC)]
    for mc in range(NCC):
        for hc in range(NHC):
            h0 = hc * HCH
            n0, nn = h0 * W, HCH * W
            # t20 psums live in the ps_15/ps_sm banks (idle by this phase) so
            # ps_mm's last user is the pipeline/t19n: the NEXT loop
            # iteration's t7/transpose matmuls get their banks much earlier.
            pool20 = ps_15 if (mc * NHC + hc) % 2 == 0 else ps_sm
            tag20 = "bank15" if (mc * NHC + hc) % 2 == 0 else "smbank"
            psum = pool20.tile([PART, HCH * W], F32, tag=tag20, name=tag20)
            nc.tensor.matmul(
                psum[:, :], (s_sb[:, mc * PART:(mc + 1) * PART]),
                (t11_sb[:, n0:n0 + nn]),
                start=True, stop=False, skip_group_check=True,
            )
            for dc in range(NCC):
                nc.tensor.matmul(
                    psum[:, :],
                    (t19n_sb[dc][:, mc * PART:(mc + 1) * PART]),
                    (t12n_sb[dc][:, n0:n0 + nn]),
                    start=False, stop=(dc == NCC - 1),
                    skip_group_check=True,
                )
            nc.vector.tensor_add(out_sb[mc][:, n0:n0 + nn],
                                 t18_sb[mc][:, n0:n0 + nn], psum[:])
            nc.sync.dma_start(out_dv[mc][:, n0:n0 + nn],
                              out_sb[mc][:, n0:n0 + nn])

    P.close_all()


_NC_CACHE = None


def _get_module():
    global _NC_CACHE
    if _NC_CACHE is None:
        _NC_CACHE = build_module()
    return _NC_CACHE


_EXEC_CACHE = {}


def _get_executor(reps=1):
    """Build the sharded PJRT executor once per module variant and cache it.

    run_bass_kernel_spmd creates fresh jit closures per call (full retrace +
    relower every time); hoisting the jit here makes repeat dispatches cheap.
    reps>1 returns an executor for the For_i timing module.
    """
    if reps in _EXEC_CACHE:
        return _EXEC_CACHE[reps]

    import jax
    from jax.sharding import Mesh, NamedSharding, PartitionSpec
    from jax.experimental.shard_map import shard_map
    from concourse.bass2jax import (
        _bass_exec_p, install_neuronx_cc_hook, partition_id_tensor)

    nc = _get_module() if reps == 1 else build_module(reps)
    install_neuronx_cc_hook()
    partition_name = nc.partition_id_tensor.name if nc.partition_id_tensor else None

    in_names, out_names, out_avals, zero_outs = [], [], [], []
    for alloc in nc.m.functions[0].allocations:
        if not isinstance(alloc, mybir.MemoryLocationSet):
            continue
        name = alloc.memorylocations[0].name
        if alloc.kind == "ExternalInput":
            if name != partition_name:
                in_names.append(name)
        elif alloc.kind == "ExternalOutput":
            out_names.append(name)
            shape = tuple(alloc.tensor_shape)
            dtype = mybir.dt.np(alloc.dtype)
            out_avals.append(jax.core.ShapedArray(shape, dtype))
            zero_outs.append(np.zeros(shape, dtype))
    n_params = len(in_names)
    n_outs = len(out_avals)
    in_names_all = in_names + out_names
    if partition_name is not None:
        in_names_all.append(partition_name)

    def _body(*args):
        operands = list(args)
        if partition_name is not None:
            operands.append(partition_id_tensor())
        return tuple(_bass_exec_p.bind(
            *operands,
            out_avals=tuple(out_avals),
            in_names=tuple(in_names_all),
            out_names=tuple(out_names),
            lowering_input_output_aliases=(),
            sim_require_finite=True,
            sim_require_nnan=True,
            nc=nc,
        ))

    devices = jax.devices()[:NCORES]
    mesh = Mesh(np.asarray(devices), ("core",))
    in_specs = (PartitionSpec("core"),) * (n_params + n_outs)
    out_specs = (PartitionSpec("core"),) * n_outs
    smapped = shard_map(_body, mesh=mesh, in_specs=in_specs,
                        out_specs=out_specs, check_rep=False)
    sharded = jax.jit(
        smapped,
        donate_argnums=tuple(range(n_params, n_params + n_outs)),
        keep_unused=True,
    )
    # non-donating variant: repeat calls on the same device-resident args
    # (the kernel writes every output element, so the zero-out operands are
    # only placeholders and need not be donated)
    sharded_nd = jax.jit(smapped, keep_unused=True)

    _EXEC_CACHE[reps] = dict(
        sharded_nd=sharded_nd,
        nc=nc, sharded=sharded, in_names=in_names, out_names=out_names,
        out_avals=out_avals, zero_outs=zero_outs, mesh=mesh,
        shard=NamedSharding(mesh, PartitionSpec("core")),
    )
    return _EXEC_CACHE[reps]


def run_in_maps(in_maps):
    """Execute the kernel for 8 per-core input dicts; returns list of out dicts."""
    ex = _get_executor()
    concat_in = [np.concatenate([np.asarray(in_maps[c][nm])
                                 for c in range(NCORES)], axis=0)
                 for nm in ex["in_names"]]
    concat_zeros = [np.zeros((NCORES * z.shape[0], *z.shape[1:]), z.dtype)
                    for z in ex["zero_outs"]]
    out_arrs = ex["sharded_nd"](*concat_in, *concat_zeros)
    return [
        {name: np.asarray(out_arrs[i]).reshape(NCORES, *ex["out_avals"][i].shape)[c]
         for i, name in enumerate(ex["out_names"])}
        for c in range(NCORES)
    ]


def _pack_cc(a):
    """[C, F] -> [128, NCC*F]: channel chunk k lands at cols [k*F,(k+1)*F)."""
    F = a.shape[1]
    return np.ascontiguousarray(
        a.reshape(NCC, PART, F).transpose(1, 0, 2).reshape(PART, NCC * F))


def _pack_T(a):
    """[C, S] -> [128, NSC*C]: s-chunk sc at cols [sc*C,(sc+1)*C), transposed
    so s sits on partitions (zero-padded to NSC*128 s rows)."""
    aT = np.zeros((NSC * PART, C), np.float32)
    aT[:S] = np.asarray(a, np.float32).reshape(C, S).T
    return np.ascontiguousarray(
        aT.reshape(NSC, PART, C).transpose(1, 0, 2).reshape(PART, NSC * C))


def prep_params(p2, w6, w7, p9, p11, w12, w15, p16):
    import ml_dtypes
    bf16 = ml_dtypes.bfloat16
    p2s = _pack_cc(np.asarray(p2, np.float32).reshape(C, S)).astype(bf16)
    p2Ts = _pack_T(p2).astype(bf16)
    w6T = _pack_cc(np.ascontiguousarray(np.asarray(w6, np.float32).T)).astype(bf16)
    w7r = np.asarray(w7, np.float32).reshape(C, C, 9).transpose(2, 1, 0)  # [ij, c, o]
    w7r = np.ascontiguousarray(w7r.reshape(9, NCC, PART, C).transpose(0, 2, 1, 3)
                               ).reshape(9 * PART, 2 * C).astype(bf16)
    p9s = np.ascontiguousarray(np.asarray(p9, np.float32).reshape(C8, S)).astype(bf16)
    p11a = np.ascontiguousarray(np.asarray(p11, np.float32).reshape(C8, 1)).astype(bf16)
    w12n = _pack_cc(-np.asarray(w12, np.float32).reshape(C, 3))
    w15T = _pack_cc(np.ascontiguousarray(np.asarray(w15, np.float32).T)
                    ).astype(bf16)
    p16T = np.ascontiguousarray(np.asarray(p16, np.float32).reshape(C, S).T
                                ).astype(bf16)  # [S, C]
    return dict(p2s=p2s, p2Ts=p2Ts, w6T=w6T, w7r=w7r, p9s=p9s, p11=p11a,
                w12n=w12n, w15T=w15T, p16T=p16T)


def kernel(x, p2, w6, w7, p9, p11, w12, w15, p16):
    params = prep_params(p2, w6, w7, p9, p11, w12, w15, p16)
    xr = np.asarray(x, np.float32).reshape(N, C, H, W)
    xa = np.zeros((N, C, H, 62), np.float32)
    xa[:, :, :, 3:3 + W] = xr
    xa = xa.reshape(N, C, H * 62)
    import ml_dtypes
    xbf = np.stack([_pack_cc(xa[n]) for n in range(N)]).astype(ml_dtypes.bfloat16)
    xTs = np.stack([_pack_T(xr[n].reshape(C, S)) for n in range(N)]
                   ).astype(ml_dtypes.bfloat16)
    in_maps = [{"xbf": xbf[n], "xTs": xTs[n], **params} for n in range(NCORES)]
    res = run_in_maps(in_maps)
    out = np.stack([res[n]["out"].astype(np.float32).reshape(C, H, W)
                    for n in range(NCORES)])
    return out

